# revision 7
# baseline (speedup 1.0000x reference)
import sys

if "/opt/trn_rl_repo" not in sys.path:
    sys.path.insert(0, "/opt/trn_rl_repo")

import numpy as np

import concourse.bass as bass
import concourse.tile as tile
from concourse import bacc, mybir
from concourse import bass_utils
from concourse.masks import make_identity

# Problem constants (hardcoded per contract).
import os
K_PAIR = int(os.environ.get("K_PAIR", "1"))
K_TRANS = os.environ.get("K_TRANS", "fp32")   # "bf16" | "fp32"
K_MMDT = os.environ.get("K_MMDT", "bf16")     # matmul weights dtype
K_BFIDENT = int(os.environ.get("K_BFIDENT", "0"))  # build bf16 identity

B = 8          # batch rows == cores
T = 4096       # tokens per core
D = 1024       # model dim
R = 128        # expert rank
NE = 8         # 7 real experts + 1 zero-weight passthrough expert
HOLE = 100000  # scatter/gather index for unoccupied slots (> T-1 -> skipped)
PAIR = K_PAIR  # 128-token tiles per indirect gather/scatter

f32 = mybir.dt.float32
bf16 = mybir.dt.bfloat16
i32 = mybir.dt.int32

_compiled = {}


def _build_nc(cap):
    tpc = cap // 128          # tiles per expert
    nslot = NE * tpc          # 128-token tiles total
    assert nslot % PAIR == 0

    nc = bacc.Bacc("TRN2", target_bir_lowering=False, debug=False)

    x_d = nc.dram_tensor("x", [T, D], f32, kind="ExternalInput").ap()
    ridx_d = nc.dram_tensor("ridx", [T, 1], i32, kind="ExternalInput").ap()
    lra_d = nc.dram_tensor("lra", [7], i32, kind="ExternalInput").ap()
    mmdt = bf16 if K_MMDT == "bf16" else f32
    wdt_d = nc.dram_tensor("wdt", [128, NE * 1024], mmdt, kind="ExternalInput").ap()
    wut_d = nc.dram_tensor("wut", [128, NE * 1024], mmdt, kind="ExternalInput").ap()
    y_d = nc.dram_tensor("y", [T, D], f32, kind="ExternalOutput").ap()

    with tile.TileContext(nc) as tc:
        with (
            tc.tile_pool(name="consts", bufs=1) as cpool,
            tc.tile_pool(name="route", bufs=1) as rpool,
            tc.tile_pool(name="weights", bufs=1) as wpool,
        ):
            # ---------- constants ----------
            ident = None
            if K_BFIDENT:
                ident = cpool.tile([128, 128], bf16)
                make_identity(nc, ident[:])
            ident_f = cpool.tile([128, 128], f32)
            make_identity(nc, ident_f[:])

            # Lstrict[p', p] = 1.0 iff p' < p  (strict partition-prefix matrix)
            lstrict = cpool.tile([128, 128], f32)
            nc.gpsimd.memset(lstrict[:], 1.0)
            nc.gpsimd.affine_select(
                out=lstrict[:], in_=lstrict[:], pattern=[[1, 128]],
                compare_op=mybir.AluOpType.is_gt, fill=0.0,
                base=0, channel_multiplier=-1,
            )

            iota128 = cpool.tile([128, 128], i32)
            nc.gpsimd.iota(iota128[:], pattern=[[1, 128]], channel_multiplier=0)
            iotaS = cpool.tile([128, nslot], i32)
            nc.gpsimd.iota(iotaS[:], pattern=[[1, nslot]], channel_multiplier=0)
            iota8 = cpool.tile([128, NE], i32)
            nc.gpsimd.iota(iota8[:], pattern=[[1, NE]], channel_multiplier=0)
            tok_i = cpool.tile([128, 32], i32)
            nc.gpsimd.iota(tok_i[:], pattern=[[1, 32]], channel_multiplier=32)
            ebase_i = cpool.tile([128, NE], i32)
            nc.gpsimd.iota(ebase_i[:], pattern=[[cap, NE]], channel_multiplier=0)

            tok_f = cpool.tile([128, 32], f32)
            nc.vector.tensor_copy(tok_f[:], tok_i[:])
            ebase_f = cpool.tile([128, NE], f32)
            nc.vector.tensor_copy(ebase_f[:], ebase_i[:])
            zeros32 = cpool.tile([128, 32], f32)
            nc.vector.memset(zeros32[:], 0.0)

            # ---------- load inputs ----------
            idx_sb = rpool.tile([128, 32], i32)  # token t = 32*p + c
            nc.sync.dma_start(idx_sb[:], ridx_d.rearrange("(p c) one -> p (c one)", p=128))

            lra_sb = rpool.tile([128, 7], i32)
            nc.gpsimd.dma_start(
                out=lra_sb[:],
                in_=bass.AP(tensor=lra_d.tensor, offset=0, ap=[[0, 128], [1, 7]]),
            )
            nc.vector.tensor_scalar_max(lra_sb[:], lra_sb[:], 0)
            nc.vector.tensor_scalar_min(lra_sb[:], lra_sb[:], 7)

            wdt_sb = wpool.tile([128, NE * 1024], mmdt)
            nc.sync.dma_start(wdt_sb[:], wdt_d)
            wut_sb = wpool.tile([128, NE * 1024], mmdt)
            nc.sync.dma_start(wut_sb[:], wut_d)

            # ---------- routing: effective expert ----------
            eq7 = rpool.tile([128, 32], i32)
            nc.vector.tensor_single_scalar(eq7[:], idx_sb[:], 7, op=mybir.AluOpType.is_equal)
            lraeq = rpool.tile([128, 32, 7], i32)
            nc.vector.tensor_tensor(
                out=lraeq[:],
                in0=idx_sb[:].unsqueeze(2).to_broadcast([128, 32, 7]),
                in1=lra_sb[:].unsqueeze(1).to_broadcast([128, 32, 7]),
                op=mybir.AluOpType.is_equal,
            )
            act = rpool.tile([128, 32], i32)
            nc.vector.tensor_reduce(act[:], lraeq[:], axis=mybir.AxisListType.X,
                                    op=mybir.AluOpType.max)
            tmp_i = rpool.tile([128, 32], i32)
            nc.vector.tensor_mul(tmp_i[:], act[:], eq7[:])
            nc.vector.tensor_sub(act[:], act[:], tmp_i[:])
            idm7 = rpool.tile([128, 32], i32)
            nc.vector.tensor_single_scalar(idm7[:], idx_sb[:], 7, op=mybir.AluOpType.subtract)
            e_eff = rpool.tile([128, 32], i32)
            nc.vector.tensor_mul(e_eff[:], act[:], idm7[:])
            nc.vector.tensor_single_scalar(e_eff[:], e_eff[:], 7, op=mybir.AluOpType.add)

            # ---------- routing: per-expert stable rank ----------
            m3 = rpool.tile([128, 32, NE], f32)
            nc.vector.tensor_tensor(
                out=m3[:],
                in0=e_eff[:].unsqueeze(2).to_broadcast([128, 32, NE]),
                in1=iota8[:].unsqueeze(1).to_broadcast([128, 32, NE]),
                op=mybir.AluOpType.is_equal,
            )
            incl = rpool.tile([128, 32, NE], f32)
            for e in range(NE):
                nc.vector.tensor_tensor_scan(
                    out=incl[:, :, e], data0=m3[:, :, e], data1=zeros32[:],
                    initial=0.0, op0=mybir.AluOpType.add, op1=mybir.AluOpType.add,
                )
            excl = rpool.tile([128, 32, NE], f32)
            nc.vector.tensor_sub(excl[:], incl[:], m3[:])
            rowtot = rpool.tile([128, NE], f32)
            nc.vector.tensor_copy(rowtot[:], incl[:, 31, :])

            with tc.tile_pool(name="psum_route", bufs=1, space="PSUM") as psr:
                pp_ps = psr.tile([128, NE], f32)
                nc.tensor.matmul(pp_ps[:], lhsT=lstrict[:], rhs=rowtot[:],
                                 start=True, stop=True)
                pp = rpool.tile([128, NE], f32)
                nc.scalar.copy(pp[:], pp_ps[:])

                pos3 = rpool.tile([128, 32, NE], f32)
                nc.vector.tensor_add(pos3[:], excl[:],
                                     pp[:].unsqueeze(1).to_broadcast([128, 32, NE]))
                nc.vector.tensor_add(pos3[:], pos3[:],
                                     ebase_f[:].unsqueeze(1).to_broadcast([128, 32, NE]))
                nc.vector.tensor_mul(pos3[:], pos3[:], m3[:])
                pos = rpool.tile([128, 32], f32)
                nc.vector.tensor_reduce(pos[:], pos3[:], axis=mybir.AxisListType.X,
                                        op=mybir.AluOpType.add)
                posi = rpool.tile([128, 32], i32)
                nc.vector.tensor_copy(posi[:], pos[:])
                b_i = rpool.tile([128, 32], i32)
                nc.vector.tensor_single_scalar(b_i[:], posi[:], 127,
                                               op=mybir.AluOpType.bitwise_and)
                a_i = rpool.tile([128, 32], i32)
                nc.vector.tensor_single_scalar(a_i[:], posi[:], 7,
                                               op=mybir.AluOpType.arith_shift_right)

                # ---------- invert permutation via one-hot matmuls ----------
                b_all = rpool.tile([128, 32, 128], f32)
                nc.vector.tensor_tensor(
                    out=b_all[:],
                    in0=b_i[:].unsqueeze(2).to_broadcast([128, 32, 128]),
                    in1=iota128[:].unsqueeze(1).to_broadcast([128, 32, 128]),
                    op=mybir.AluOpType.is_equal,
                )
                aw = rpool.tile([128, 32, 2 * nslot], f32)
                nc.vector.tensor_tensor(
                    out=aw[:, :, 0:nslot],
                    in0=a_i[:].unsqueeze(2).to_broadcast([128, 32, nslot]),
                    in1=iotaS[:].unsqueeze(1).to_broadcast([128, 32, nslot]),
                    op=mybir.AluOpType.is_equal,
                )
                nc.vector.tensor_tensor(
                    out=aw[:, :, nslot:2 * nslot],
                    in0=aw[:, :, 0:nslot],
                    in1=tok_f[:].unsqueeze(2).to_broadcast([128, 32, nslot]),
                    op=mybir.AluOpType.mult,
                )

                out_ps = psr.tile([128, 2 * nslot], f32)
                for c in range(32):
                    nc.tensor.matmul(out_ps[:], lhsT=b_all[:, c, :], rhs=aw[:, c, :],
                                     start=(c == 0), stop=(c == 31))
                out_sb = rpool.tile([128, 2 * nslot], f32)
                nc.scalar.copy(out_sb[:], out_ps[:])

            idxf = rpool.tile([128, nslot], f32)
            nc.vector.tensor_single_scalar(idxf[:], out_sb[:, 0:nslot], float(-HOLE),
                                           op=mybir.AluOpType.mult)
            nc.vector.tensor_add(idxf[:], idxf[:], out_sb[:, nslot:2 * nslot])
            nc.vector.tensor_scalar_add(idxf[:], idxf[:], float(HOLE))
            idx32 = rpool.tile([128, nslot], i32)
            nc.vector.tensor_copy(idx32[:], idxf[:])

            # ---------- main loop ----------
            with (
                tc.tile_pool(name="gpool", bufs=4) as gpool,
                tc.tile_pool(name="gbpool", bufs=3) as gbpool,
                tc.tile_pool(name="gtpool", bufs=4) as gtpool,
                tc.tile_pool(name="dpool", bufs=3) as dpool,
                tc.tile_pool(name="ypool", bufs=4) as ypool,
                tc.tile_pool(name="psA", bufs=3, space="PSUM") as psA,
                tc.tile_pool(name="psD", bufs=2, space="PSUM") as psD,
            ):
                for s in range(nslot):
                    e = s // tpc
                    off = idx32[:, s:s + 1]

                    g = gpool.tile([128, D], f32)
                    nc.gpsimd.indirect_dma_start(
                        out=g[:], out_offset=None,
                        in_=x_d,
                        in_offset=bass.IndirectOffsetOnAxis(ap=off, axis=0),
                        bounds_check=T - 1, oob_is_err=False,
                    )

                    if K_TRANS == "bf16":
                        gb = gbpool.tile([128, D], bf16)
                        nc.vector.tensor_copy(gb[:], g[:])
                        gt_ps = psA.tile([128, D], bf16, tag="gtu")
                        tsrc, tident = gb, ident
                    else:
                        gt_ps = psA.tile([128, D], f32, tag="gtu")
                        tsrc, tident = g, ident_f
                    for k in range(8):
                        nc.tensor.transpose(
                            out=gt_ps[:, k * 128:(k + 1) * 128],
                            in_=tsrc[:, k * 128:(k + 1) * 128],
                            identity=tident[:],
                        )
                    gt_sb = gtpool.tile([128, D], mmdt)
                    nc.scalar.copy(gt_sb[:, 0:512], gt_ps[:, 0:512])
                    nc.vector.tensor_copy(gt_sb[:, 512:1024], gt_ps[:, 512:1024])

                    d_ps = psD.tile([128, 128], f32)
                    for k in range(8):
                        nc.tensor.matmul(
                            d_ps[:],
                            lhsT=wdt_sb[:, e * 1024 + k * 128: e * 1024 + (k + 1) * 128],
                            rhs=gt_sb[:, k * 128:(k + 1) * 128],
                            start=(k == 0), stop=(k == 7),
                        )
                    d_sb = dpool.tile([128, 128], mmdt)
                    nc.scalar.copy(d_sb[:], d_ps[:])

                    u_ps = psA.tile([128, D], f32, tag="gtu")
                    nc.tensor.matmul(u_ps[:, 0:512], lhsT=d_sb[:],
                                     rhs=wut_sb[:, e * 1024: e * 1024 + 512],
                                     start=True, stop=True)
                    nc.tensor.matmul(u_ps[:, 512:1024], lhsT=d_sb[:],
                                     rhs=wut_sb[:, e * 1024 + 512: e * 1024 + 1024],
                                     start=True, stop=True)

                    y_sb = ypool.tile([128, D], f32)
                    nc.vector.tensor_add(y_sb[:], u_ps[:], g[:])

                    nc.gpsimd.indirect_dma_start(
                        out=y_d, out_offset=bass.IndirectOffsetOnAxis(ap=off, axis=0),
                        in_=y_sb[:], in_offset=None,
                        bounds_check=T - 1, oob_is_err=False,
                    )

    nc.compile()
    return nc


def _prep_weights(Wd: np.ndarray, Wu: np.ndarray):
    import ml_dtypes
    wdt = np.zeros((128, NE, 8, 128), dtype=np.float32)
    wdt[:, :7] = Wd.reshape(7, 128, 8, 128).transpose(3, 0, 2, 1)
    wut = np.zeros((128, NE, 1024), dtype=np.float32)
    wut[:, :7] = Wu.transpose(2, 0, 1)
    odt = ml_dtypes.bfloat16 if K_MMDT == "bf16" else np.float32
    return (np.ascontiguousarray(wdt.reshape(128, NE * 1024)).astype(odt),
            np.ascontiguousarray(wut.reshape(128, NE * 1024)).astype(odt))


def _pick_cap(router_indices: np.ndarray, LRA_mask: np.ndarray) -> int:
    tok = router_indices[..., 0]
    key_active = np.zeros(8, dtype=bool)
    key_active[np.clip(LRA_mask, 0, 7)] = True
    active = key_active[tok] & (tok != 7)
    e_eff = np.where(active, tok, 7)
    mx = 1
    for b in range(e_eff.shape[0]):
        mx = max(mx, int(np.bincount(e_eff[b], minlength=8).max()))
    cap = ((mx + 127) // 128) * 128
    # keep slot count PAIR-aligned
    while (NE * cap // 128) % PAIR:
        cap += 128
    return cap


def kernel(x, router_indices, LRA_mask, Wd, Wu):
    x = np.asarray(x, dtype=np.float32)
    router_indices = np.asarray(router_indices, dtype=np.int32)
    LRA_mask = np.asarray(LRA_mask, dtype=np.int32)
    Wd = np.asarray(Wd, dtype=np.float32)
    Wu = np.asarray(Wu, dtype=np.float32)

    cap = _pick_cap(router_indices, LRA_mask)
    if cap not in _compiled:
        _compiled[cap] = _build_nc(cap)
    nc = _compiled[cap]

    wdt, wut = _prep_weights(Wd, Wu)
    in_maps = []
    for b in range(B):
        in_maps.append({
            "x": np.ascontiguousarray(x[b]),
            "ridx": np.ascontiguousarray(router_indices[b]),
            "lra": LRA_mask,
            "wdt": wdt,
            "wut": wut,
        })
    res = bass_utils.run_bass_kernel_spmd(nc, in_maps, core_ids=list(range(B)))
    out = np.stack([res.results[b]["y"] for b in range(B)], axis=0)
    return out.astype(np.float32)


if __name__ == "__main__":
    rng = np.random.default_rng(0)
    x = rng.standard_normal((B, T, D), dtype=np.float32)
    ridx = rng.integers(0, 8, size=(B, T, 1), dtype=np.int32)
    lra = np.arange(7, dtype=np.int32)
    Wd = (rng.standard_normal((7, 128, 1024)) * 0.01).astype(np.float32)
    Wu = (rng.standard_normal((7, 1024, 128)) * 0.01).astype(np.float32)
    y = kernel(x, ridx, lra, Wd, Wu)
    print("out", y.shape, y.dtype)


# revision 8
# speedup vs baseline: 1.0478x; 1.0478x over previous
import sys

if "/opt/trn_rl_repo" not in sys.path:
    sys.path.insert(0, "/opt/trn_rl_repo")

import numpy as np

import concourse.bass as bass
import concourse.tile as tile
from concourse import bacc, mybir
from concourse import bass_utils
from concourse.masks import make_identity

# Problem constants (hardcoded per contract).
import os
K_PAIR = int(os.environ.get("K_PAIR", "1"))
K_TRANS = os.environ.get("K_TRANS", "bf16")   # "bf16" | "fp32"
K_MMDT = os.environ.get("K_MMDT", "bf16")     # matmul weights dtype
K_BFIDENT = int(os.environ.get("K_BFIDENT", "1"))  # build bf16 identity

B = 8          # batch rows == cores
T = 4096       # tokens per core
D = 1024       # model dim
R = 128        # expert rank
NE = 8         # 7 real experts + 1 zero-weight passthrough expert
HOLE = 100000  # scatter/gather index for unoccupied slots (> T-1 -> skipped)
PAIR = K_PAIR  # 128-token tiles per indirect gather/scatter

f32 = mybir.dt.float32
bf16 = mybir.dt.bfloat16
i32 = mybir.dt.int32

_compiled = {}


def _build_nc(cap):
    tpc = cap // 128          # tiles per expert
    nslot = NE * tpc          # 128-token tiles total
    assert nslot % PAIR == 0

    nc = bacc.Bacc("TRN2", target_bir_lowering=False, debug=False)

    x_d = nc.dram_tensor("x", [T, D], f32, kind="ExternalInput").ap()
    ridx_d = nc.dram_tensor("ridx", [T, 1], i32, kind="ExternalInput").ap()
    lra_d = nc.dram_tensor("lra", [7], i32, kind="ExternalInput").ap()
    mmdt = bf16 if K_MMDT == "bf16" else f32
    wdt_d = nc.dram_tensor("wdt", [128, NE * 1024], mmdt, kind="ExternalInput").ap()
    wut_d = nc.dram_tensor("wut", [128, NE * 1024], mmdt, kind="ExternalInput").ap()
    y_d = nc.dram_tensor("y", [T, D], f32, kind="ExternalOutput").ap()

    with tile.TileContext(nc) as tc:
        with (
            tc.tile_pool(name="consts", bufs=1) as cpool,
            tc.tile_pool(name="route", bufs=1) as rpool,
            tc.tile_pool(name="weights", bufs=1) as wpool,
        ):
            # ---------- constants ----------
            ident = None
            if K_BFIDENT:
                ident = cpool.tile([128, 128], bf16)
                make_identity(nc, ident[:])
            ident_f = cpool.tile([128, 128], f32)
            make_identity(nc, ident_f[:])

            # Lstrict[p', p] = 1.0 iff p' < p  (strict partition-prefix matrix)
            lstrict = cpool.tile([128, 128], f32)
            nc.gpsimd.memset(lstrict[:], 1.0)
            nc.gpsimd.affine_select(
                out=lstrict[:], in_=lstrict[:], pattern=[[1, 128]],
                compare_op=mybir.AluOpType.is_gt, fill=0.0,
                base=0, channel_multiplier=-1,
            )

            iota128 = cpool.tile([128, 128], i32)
            nc.gpsimd.iota(iota128[:], pattern=[[1, 128]], channel_multiplier=0)
            iotaS = cpool.tile([128, nslot], i32)
            nc.gpsimd.iota(iotaS[:], pattern=[[1, nslot]], channel_multiplier=0)
            iota8 = cpool.tile([128, NE], i32)
            nc.gpsimd.iota(iota8[:], pattern=[[1, NE]], channel_multiplier=0)
            tok_i = cpool.tile([128, 32], i32)
            nc.gpsimd.iota(tok_i[:], pattern=[[1, 32]], channel_multiplier=32)
            ebase_i = cpool.tile([128, NE], i32)
            nc.gpsimd.iota(ebase_i[:], pattern=[[cap, NE]], channel_multiplier=0)

            tok_f = cpool.tile([128, 32], f32)
            nc.vector.tensor_copy(tok_f[:], tok_i[:])
            ebase_f = cpool.tile([128, NE], f32)
            nc.vector.tensor_copy(ebase_f[:], ebase_i[:])
            zeros32 = cpool.tile([128, 32], f32)
            nc.vector.memset(zeros32[:], 0.0)

            # ---------- load inputs ----------
            idx_sb = rpool.tile([128, 32], i32)  # token t = 32*p + c
            nc.sync.dma_start(idx_sb[:], ridx_d.rearrange("(p c) one -> p (c one)", p=128))

            lra_sb = rpool.tile([128, 7], i32)
            nc.gpsimd.dma_start(
                out=lra_sb[:],
                in_=bass.AP(tensor=lra_d.tensor, offset=0, ap=[[0, 128], [1, 7]]),
            )
            nc.vector.tensor_scalar_max(lra_sb[:], lra_sb[:], 0)
            nc.vector.tensor_scalar_min(lra_sb[:], lra_sb[:], 7)

            wdt_sb = wpool.tile([128, NE * 1024], mmdt)
            nc.sync.dma_start(wdt_sb[:], wdt_d)
            wut_sb = wpool.tile([128, NE * 1024], mmdt)
            nc.sync.dma_start(wut_sb[:], wut_d)

            # ---------- routing: effective expert ----------
            eq7 = rpool.tile([128, 32], i32)
            nc.vector.tensor_single_scalar(eq7[:], idx_sb[:], 7, op=mybir.AluOpType.is_equal)
            lraeq = rpool.tile([128, 32, 7], i32)
            nc.vector.tensor_tensor(
                out=lraeq[:],
                in0=idx_sb[:].unsqueeze(2).to_broadcast([128, 32, 7]),
                in1=lra_sb[:].unsqueeze(1).to_broadcast([128, 32, 7]),
                op=mybir.AluOpType.is_equal,
            )
            act = rpool.tile([128, 32], i32)
            nc.vector.tensor_reduce(act[:], lraeq[:], axis=mybir.AxisListType.X,
                                    op=mybir.AluOpType.max)
            tmp_i = rpool.tile([128, 32], i32)
            nc.vector.tensor_mul(tmp_i[:], act[:], eq7[:])
            nc.vector.tensor_sub(act[:], act[:], tmp_i[:])
            idm7 = rpool.tile([128, 32], i32)
            nc.vector.tensor_single_scalar(idm7[:], idx_sb[:], 7, op=mybir.AluOpType.subtract)
            e_eff = rpool.tile([128, 32], i32)
            nc.vector.tensor_mul(e_eff[:], act[:], idm7[:])
            nc.vector.tensor_single_scalar(e_eff[:], e_eff[:], 7, op=mybir.AluOpType.add)

            # ---------- routing: per-expert stable rank ----------
            m3 = rpool.tile([128, 32, NE], f32)
            nc.vector.tensor_tensor(
                out=m3[:],
                in0=e_eff[:].unsqueeze(2).to_broadcast([128, 32, NE]),
                in1=iota8[:].unsqueeze(1).to_broadcast([128, 32, NE]),
                op=mybir.AluOpType.is_equal,
            )
            incl = rpool.tile([128, 32, NE], f32)
            for e in range(NE):
                nc.vector.tensor_tensor_scan(
                    out=incl[:, :, e], data0=m3[:, :, e], data1=zeros32[:],
                    initial=0.0, op0=mybir.AluOpType.add, op1=mybir.AluOpType.add,
                )
            excl = rpool.tile([128, 32, NE], f32)
            nc.vector.tensor_sub(excl[:], incl[:], m3[:])
            rowtot = rpool.tile([128, NE], f32)
            nc.vector.tensor_copy(rowtot[:], incl[:, 31, :])

            with tc.tile_pool(name="psum_route", bufs=1, space="PSUM") as psr:
                pp_ps = psr.tile([128, NE], f32)
                nc.tensor.matmul(pp_ps[:], lhsT=lstrict[:], rhs=rowtot[:],
                                 start=True, stop=True)
                pp = rpool.tile([128, NE], f32)
                nc.scalar.copy(pp[:], pp_ps[:])

                pos3 = rpool.tile([128, 32, NE], f32)
                nc.vector.tensor_add(pos3[:], excl[:],
                                     pp[:].unsqueeze(1).to_broadcast([128, 32, NE]))
                nc.vector.tensor_add(pos3[:], pos3[:],
                                     ebase_f[:].unsqueeze(1).to_broadcast([128, 32, NE]))
                nc.vector.tensor_mul(pos3[:], pos3[:], m3[:])
                pos = rpool.tile([128, 32], f32)
                nc.vector.tensor_reduce(pos[:], pos3[:], axis=mybir.AxisListType.X,
                                        op=mybir.AluOpType.add)
                posi = rpool.tile([128, 32], i32)
                nc.vector.tensor_copy(posi[:], pos[:])
                b_i = rpool.tile([128, 32], i32)
                nc.vector.tensor_single_scalar(b_i[:], posi[:], 127,
                                               op=mybir.AluOpType.bitwise_and)
                a_i = rpool.tile([128, 32], i32)
                nc.vector.tensor_single_scalar(a_i[:], posi[:], 7,
                                               op=mybir.AluOpType.arith_shift_right)

                # ---------- invert permutation via one-hot matmuls ----------
                b_all = rpool.tile([128, 32, 128], f32)
                nc.vector.tensor_tensor(
                    out=b_all[:],
                    in0=b_i[:].unsqueeze(2).to_broadcast([128, 32, 128]),
                    in1=iota128[:].unsqueeze(1).to_broadcast([128, 32, 128]),
                    op=mybir.AluOpType.is_equal,
                )
                aw = rpool.tile([128, 32, 2 * nslot], f32)
                nc.vector.tensor_tensor(
                    out=aw[:, :, 0:nslot],
                    in0=a_i[:].unsqueeze(2).to_broadcast([128, 32, nslot]),
                    in1=iotaS[:].unsqueeze(1).to_broadcast([128, 32, nslot]),
                    op=mybir.AluOpType.is_equal,
                )
                nc.vector.tensor_tensor(
                    out=aw[:, :, nslot:2 * nslot],
                    in0=aw[:, :, 0:nslot],
                    in1=tok_f[:].unsqueeze(2).to_broadcast([128, 32, nslot]),
                    op=mybir.AluOpType.mult,
                )

                out_ps = psr.tile([128, 2 * nslot], f32)
                for c in range(32):
                    nc.tensor.matmul(out_ps[:], lhsT=b_all[:, c, :], rhs=aw[:, c, :],
                                     start=(c == 0), stop=(c == 31))
                out_sb = rpool.tile([128, 2 * nslot], f32)
                nc.scalar.copy(out_sb[:], out_ps[:])

            idxf = rpool.tile([128, nslot], f32)
            nc.vector.tensor_single_scalar(idxf[:], out_sb[:, 0:nslot], float(-HOLE),
                                           op=mybir.AluOpType.mult)
            nc.vector.tensor_add(idxf[:], idxf[:], out_sb[:, nslot:2 * nslot])
            nc.vector.tensor_scalar_add(idxf[:], idxf[:], float(HOLE))
            idx32 = rpool.tile([128, nslot], i32)
            nc.vector.tensor_copy(idx32[:], idxf[:])

            # ---------- main loop ----------
            with (
                tc.tile_pool(name="gpool", bufs=4) as gpool,
                tc.tile_pool(name="gbpool", bufs=3) as gbpool,
                tc.tile_pool(name="gtpool", bufs=4) as gtpool,
                tc.tile_pool(name="dpool", bufs=3) as dpool,
                tc.tile_pool(name="ypool", bufs=4) as ypool,
                tc.tile_pool(name="psGT", bufs=2, space="PSUM") as psGT,
                tc.tile_pool(name="psU", bufs=2, space="PSUM") as psU,
                tc.tile_pool(name="psD", bufs=2, space="PSUM") as psD,
            ):
                for s in range(nslot):
                    e = s // tpc
                    off = idx32[:, s:s + 1]

                    g = gpool.tile([128, D], f32)
                    nc.gpsimd.indirect_dma_start(
                        out=g[:], out_offset=None,
                        in_=x_d,
                        in_offset=bass.IndirectOffsetOnAxis(ap=off, axis=0),
                        bounds_check=T - 1, oob_is_err=False,
                    )

                    if K_TRANS == "bf16":
                        gb = gbpool.tile([128, D], bf16)
                        nc.vector.tensor_copy(gb[:], g[:])
                        gt_ps = psGT.tile([128, D], bf16)
                        tsrc, tident = gb, ident
                    else:
                        gt_ps = psGT.tile([128, D], f32)
                        tsrc, tident = g, ident_f
                    for k in range(8):
                        nc.tensor.transpose(
                            out=gt_ps[:, k * 128:(k + 1) * 128],
                            in_=tsrc[:, k * 128:(k + 1) * 128],
                            identity=tident[:],
                        )
                    gt_sb = gtpool.tile([128, D], mmdt)
                    nc.scalar.copy(gt_sb[:, 0:512], gt_ps[:, 0:512])
                    nc.scalar.copy(gt_sb[:, 512:1024], gt_ps[:, 512:1024])

                    d_ps = psD.tile([128, 128], f32)
                    for k in range(8):
                        nc.tensor.matmul(
                            d_ps[:],
                            lhsT=wdt_sb[:, e * 1024 + k * 128: e * 1024 + (k + 1) * 128],
                            rhs=gt_sb[:, k * 128:(k + 1) * 128],
                            start=(k == 0), stop=(k == 7),
                        )
                    d_sb = dpool.tile([128, 128], mmdt)
                    nc.scalar.copy(d_sb[:], d_ps[:])

                    u_ps = psU.tile([128, D], f32)
                    nc.tensor.matmul(u_ps[:, 0:512], lhsT=d_sb[:],
                                     rhs=wut_sb[:, e * 1024: e * 1024 + 512],
                                     start=True, stop=True)
                    nc.tensor.matmul(u_ps[:, 512:1024], lhsT=d_sb[:],
                                     rhs=wut_sb[:, e * 1024 + 512: e * 1024 + 1024],
                                     start=True, stop=True)

                    y_sb = ypool.tile([128, D], f32)
                    nc.vector.tensor_add(y_sb[:], u_ps[:], g[:])

                    nc.gpsimd.indirect_dma_start(
                        out=y_d, out_offset=bass.IndirectOffsetOnAxis(ap=off, axis=0),
                        in_=y_sb[:], in_offset=None,
                        bounds_check=T - 1, oob_is_err=False,
                    )

    nc.compile()
    return nc


def _prep_weights(Wd: np.ndarray, Wu: np.ndarray):
    import ml_dtypes
    wdt = np.zeros((128, NE, 8, 128), dtype=np.float32)
    wdt[:, :7] = Wd.reshape(7, 128, 8, 128).transpose(3, 0, 2, 1)
    wut = np.zeros((128, NE, 1024), dtype=np.float32)
    wut[:, :7] = Wu.transpose(2, 0, 1)
    odt = ml_dtypes.bfloat16 if K_MMDT == "bf16" else np.float32
    return (np.ascontiguousarray(wdt.reshape(128, NE * 1024)).astype(odt),
            np.ascontiguousarray(wut.reshape(128, NE * 1024)).astype(odt))


def _pick_cap(router_indices: np.ndarray, LRA_mask: np.ndarray) -> int:
    tok = router_indices[..., 0]
    key_active = np.zeros(8, dtype=bool)
    key_active[np.clip(LRA_mask, 0, 7)] = True
    active = key_active[tok] & (tok != 7)
    e_eff = np.where(active, tok, 7)
    mx = 1
    for b in range(e_eff.shape[0]):
        mx = max(mx, int(np.bincount(e_eff[b], minlength=8).max()))
    cap = ((mx + 127) // 128) * 128
    # keep slot count PAIR-aligned
    while (NE * cap // 128) % PAIR:
        cap += 128
    return cap


def kernel(x, router_indices, LRA_mask, Wd, Wu):
    x = np.asarray(x, dtype=np.float32)
    router_indices = np.asarray(router_indices, dtype=np.int32)
    LRA_mask = np.asarray(LRA_mask, dtype=np.int32)
    Wd = np.asarray(Wd, dtype=np.float32)
    Wu = np.asarray(Wu, dtype=np.float32)

    cap = _pick_cap(router_indices, LRA_mask)
    if cap not in _compiled:
        _compiled[cap] = _build_nc(cap)
    nc = _compiled[cap]

    wdt, wut = _prep_weights(Wd, Wu)
    in_maps = []
    for b in range(B):
        in_maps.append({
            "x": np.ascontiguousarray(x[b]),
            "ridx": np.ascontiguousarray(router_indices[b]),
            "lra": LRA_mask,
            "wdt": wdt,
            "wut": wut,
        })
    res = bass_utils.run_bass_kernel_spmd(nc, in_maps, core_ids=list(range(B)))
    out = np.stack([res.results[b]["y"] for b in range(B)], axis=0)
    return out.astype(np.float32)


if __name__ == "__main__":
    rng = np.random.default_rng(0)
    x = rng.standard_normal((B, T, D), dtype=np.float32)
    ridx = rng.integers(0, 8, size=(B, T, 1), dtype=np.int32)
    lra = np.arange(7, dtype=np.int32)
    Wd = (rng.standard_normal((7, 128, 1024)) * 0.01).astype(np.float32)
    Wu = (rng.standard_normal((7, 1024, 128)) * 0.01).astype(np.float32)
    y = kernel(x, ridx, lra, Wd, Wu)
    print("out", y.shape, y.dtype)


# revision 9
# speedup vs baseline: 1.3679x; 1.3056x over previous
import sys

if "/opt/trn_rl_repo" not in sys.path:
    sys.path.insert(0, "/opt/trn_rl_repo")

import numpy as np

import concourse.bass as bass
import concourse.tile as tile
from concourse import bacc, mybir
from concourse import bass_utils
from concourse.masks import make_identity

# Problem constants (hardcoded per contract).
import os
K_PAIR = int(os.environ.get("K_PAIR", "2"))
K_TRANS = os.environ.get("K_TRANS", "bf16")   # "bf16" | "fp32"
K_MMDT = os.environ.get("K_MMDT", "bf16")     # matmul weights dtype
K_BFIDENT = int(os.environ.get("K_BFIDENT", "1"))  # build bf16 identity

B = 8          # batch rows == cores
T = 4096       # tokens per core
D = 1024       # model dim
R = 128        # expert rank
NE = 8         # 7 real experts + 1 zero-weight passthrough expert
HOLE = 100000  # scatter/gather index for unoccupied slots (> T-1 -> skipped)
PAIR = K_PAIR  # 128-token tiles per indirect gather/scatter

f32 = mybir.dt.float32
bf16 = mybir.dt.bfloat16
i32 = mybir.dt.int32

_compiled = {}


def _build_nc(cap):
    tpc = cap // 128          # tiles per expert
    nslot = NE * tpc          # 128-token tiles total
    assert nslot % PAIR == 0

    nc = bacc.Bacc("TRN2", target_bir_lowering=False, debug=False)

    x_d = nc.dram_tensor("x", [T, D], f32, kind="ExternalInput").ap()
    ridx_d = nc.dram_tensor("ridx", [T, 1], i32, kind="ExternalInput").ap()
    lra_d = nc.dram_tensor("lra", [7], i32, kind="ExternalInput").ap()
    mmdt = bf16 if K_MMDT == "bf16" else f32
    wdt_d = nc.dram_tensor("wdt", [128, NE * 1024], mmdt, kind="ExternalInput").ap()
    wut_d = nc.dram_tensor("wut", [128, NE * 1024], mmdt, kind="ExternalInput").ap()
    y_d = nc.dram_tensor("y", [T, D], f32, kind="ExternalOutput").ap()

    with tile.TileContext(nc) as tc:
        with (
            tc.tile_pool(name="consts", bufs=1) as cpool,
            tc.tile_pool(name="route", bufs=1) as rpool,
            tc.tile_pool(name="weights", bufs=1) as wpool,
        ):
            # ---------- constants ----------
            ident = None
            if K_BFIDENT:
                ident = cpool.tile([128, 128], bf16)
                make_identity(nc, ident[:])
            ident_f = cpool.tile([128, 128], f32)
            make_identity(nc, ident_f[:])

            # Lstrict[p', p] = 1.0 iff p' < p  (strict partition-prefix matrix)
            lstrict = cpool.tile([128, 128], f32)
            nc.gpsimd.memset(lstrict[:], 1.0)
            nc.gpsimd.affine_select(
                out=lstrict[:], in_=lstrict[:], pattern=[[1, 128]],
                compare_op=mybir.AluOpType.is_gt, fill=0.0,
                base=0, channel_multiplier=-1,
            )

            iota128 = cpool.tile([128, 128], i32)
            nc.gpsimd.iota(iota128[:], pattern=[[1, 128]], channel_multiplier=0)
            iotaS = cpool.tile([128, nslot], i32)
            nc.gpsimd.iota(iotaS[:], pattern=[[1, nslot]], channel_multiplier=0)
            iota8 = cpool.tile([128, NE], i32)
            nc.gpsimd.iota(iota8[:], pattern=[[1, NE]], channel_multiplier=0)
            tok_i = cpool.tile([128, 32], i32)
            nc.gpsimd.iota(tok_i[:], pattern=[[1, 32]], channel_multiplier=32)
            ebase_i = cpool.tile([128, NE], i32)
            nc.gpsimd.iota(ebase_i[:], pattern=[[cap, NE]], channel_multiplier=0)

            tok_f = cpool.tile([128, 32], f32)
            nc.vector.tensor_copy(tok_f[:], tok_i[:])
            ebase_f = cpool.tile([128, NE], f32)
            nc.vector.tensor_copy(ebase_f[:], ebase_i[:])
            zeros32 = cpool.tile([128, 32], f32)
            nc.vector.memset(zeros32[:], 0.0)

            # ---------- load inputs ----------
            idx_sb = rpool.tile([128, 32], i32)  # token t = 32*p + c
            nc.sync.dma_start(idx_sb[:], ridx_d.rearrange("(p c) one -> p (c one)", p=128))

            lra_sb = rpool.tile([128, 7], i32)
            nc.gpsimd.dma_start(
                out=lra_sb[:],
                in_=bass.AP(tensor=lra_d.tensor, offset=0, ap=[[0, 128], [1, 7]]),
            )
            nc.vector.tensor_scalar_max(lra_sb[:], lra_sb[:], 0)
            nc.vector.tensor_scalar_min(lra_sb[:], lra_sb[:], 7)

            wdt_sb = wpool.tile([128, NE * 1024], mmdt)
            nc.sync.dma_start(wdt_sb[:], wdt_d)
            wut_sb = wpool.tile([128, NE * 1024], mmdt)
            nc.sync.dma_start(wut_sb[:], wut_d)

            # ---------- routing: effective expert ----------
            eq7 = rpool.tile([128, 32], i32)
            nc.vector.tensor_single_scalar(eq7[:], idx_sb[:], 7, op=mybir.AluOpType.is_equal)
            lraeq = rpool.tile([128, 32, 7], i32)
            nc.vector.tensor_tensor(
                out=lraeq[:],
                in0=idx_sb[:].unsqueeze(2).to_broadcast([128, 32, 7]),
                in1=lra_sb[:].unsqueeze(1).to_broadcast([128, 32, 7]),
                op=mybir.AluOpType.is_equal,
            )
            act = rpool.tile([128, 32], i32)
            nc.vector.tensor_reduce(act[:], lraeq[:], axis=mybir.AxisListType.X,
                                    op=mybir.AluOpType.max)
            tmp_i = rpool.tile([128, 32], i32)
            nc.vector.tensor_mul(tmp_i[:], act[:], eq7[:])
            nc.vector.tensor_sub(act[:], act[:], tmp_i[:])
            idm7 = rpool.tile([128, 32], i32)
            nc.vector.tensor_single_scalar(idm7[:], idx_sb[:], 7, op=mybir.AluOpType.subtract)
            e_eff = rpool.tile([128, 32], i32)
            nc.vector.tensor_mul(e_eff[:], act[:], idm7[:])
            nc.vector.tensor_single_scalar(e_eff[:], e_eff[:], 7, op=mybir.AluOpType.add)

            # ---------- routing: per-expert stable rank ----------
            m3 = rpool.tile([128, 32, NE], f32)
            nc.vector.tensor_tensor(
                out=m3[:],
                in0=e_eff[:].unsqueeze(2).to_broadcast([128, 32, NE]),
                in1=iota8[:].unsqueeze(1).to_broadcast([128, 32, NE]),
                op=mybir.AluOpType.is_equal,
            )
            incl = rpool.tile([128, 32, NE], f32)
            for e in range(NE):
                nc.vector.tensor_tensor_scan(
                    out=incl[:, :, e], data0=m3[:, :, e], data1=zeros32[:],
                    initial=0.0, op0=mybir.AluOpType.add, op1=mybir.AluOpType.add,
                )
            excl = rpool.tile([128, 32, NE], f32)
            nc.vector.tensor_sub(excl[:], incl[:], m3[:])
            rowtot = rpool.tile([128, NE], f32)
            nc.vector.tensor_copy(rowtot[:], incl[:, 31, :])

            with tc.tile_pool(name="psum_route", bufs=1, space="PSUM") as psr:
                pp_ps = psr.tile([128, NE], f32)
                nc.tensor.matmul(pp_ps[:], lhsT=lstrict[:], rhs=rowtot[:],
                                 start=True, stop=True)
                pp = rpool.tile([128, NE], f32)
                nc.scalar.copy(pp[:], pp_ps[:])

                pos3 = rpool.tile([128, 32, NE], f32)
                nc.vector.tensor_add(pos3[:], excl[:],
                                     pp[:].unsqueeze(1).to_broadcast([128, 32, NE]))
                nc.vector.tensor_add(pos3[:], pos3[:],
                                     ebase_f[:].unsqueeze(1).to_broadcast([128, 32, NE]))
                nc.vector.tensor_mul(pos3[:], pos3[:], m3[:])
                pos = rpool.tile([128, 32], f32)
                nc.vector.tensor_reduce(pos[:], pos3[:], axis=mybir.AxisListType.X,
                                        op=mybir.AluOpType.add)
                posi = rpool.tile([128, 32], i32)
                nc.vector.tensor_copy(posi[:], pos[:])
                b_i = rpool.tile([128, 32], i32)
                nc.vector.tensor_single_scalar(b_i[:], posi[:], 127,
                                               op=mybir.AluOpType.bitwise_and)
                a_i = rpool.tile([128, 32], i32)
                nc.vector.tensor_single_scalar(a_i[:], posi[:], 7,
                                               op=mybir.AluOpType.arith_shift_right)

                # ---------- invert permutation via one-hot matmuls ----------
                b_all = rpool.tile([128, 32, 128], f32)
                nc.vector.tensor_tensor(
                    out=b_all[:],
                    in0=b_i[:].unsqueeze(2).to_broadcast([128, 32, 128]),
                    in1=iota128[:].unsqueeze(1).to_broadcast([128, 32, 128]),
                    op=mybir.AluOpType.is_equal,
                )
                aw = rpool.tile([128, 32, 2 * nslot], f32)
                nc.vector.tensor_tensor(
                    out=aw[:, :, 0:nslot],
                    in0=a_i[:].unsqueeze(2).to_broadcast([128, 32, nslot]),
                    in1=iotaS[:].unsqueeze(1).to_broadcast([128, 32, nslot]),
                    op=mybir.AluOpType.is_equal,
                )
                nc.vector.tensor_tensor(
                    out=aw[:, :, nslot:2 * nslot],
                    in0=aw[:, :, 0:nslot],
                    in1=tok_f[:].unsqueeze(2).to_broadcast([128, 32, nslot]),
                    op=mybir.AluOpType.mult,
                )

                out_ps = psr.tile([128, 2 * nslot], f32)
                for c in range(32):
                    nc.tensor.matmul(out_ps[:], lhsT=b_all[:, c, :], rhs=aw[:, c, :],
                                     start=(c == 0), stop=(c == 31))
                out_sb = rpool.tile([128, 2 * nslot], f32)
                nc.scalar.copy(out_sb[:], out_ps[:])

            idxf = rpool.tile([128, nslot], f32)
            nc.vector.tensor_single_scalar(idxf[:], out_sb[:, 0:nslot], float(-HOLE),
                                           op=mybir.AluOpType.mult)
            nc.vector.tensor_add(idxf[:], idxf[:], out_sb[:, nslot:2 * nslot])
            nc.vector.tensor_scalar_add(idxf[:], idxf[:], float(HOLE))
            idx32 = rpool.tile([128, nslot], i32)
            nc.vector.tensor_copy(idx32[:], idxf[:])

            # ---------- main loop ----------
            with (
                tc.tile_pool(name="gpool", bufs=4) as gpool,
                tc.tile_pool(name="gbpool", bufs=4) as gbpool,
                tc.tile_pool(name="gtpool", bufs=6) as gtpool,
                tc.tile_pool(name="dpool", bufs=6) as dpool,
                tc.tile_pool(name="ypool", bufs=4) as ypool,
                tc.tile_pool(name="psGT", bufs=2, space="PSUM") as psGT,
                tc.tile_pool(name="psU", bufs=2, space="PSUM") as psU,
                tc.tile_pool(name="psD", bufs=2, space="PSUM") as psD,
            ):
                GRP = K_PAIR
                for grp in range(nslot // GRP):
                    offs = idx32[:, grp * GRP:(grp + 1) * GRP]
                    g2 = gpool.tile([128, GRP * D], f32)
                    nc.gpsimd.indirect_dma_start(
                        out=g2[:], out_offset=None,
                        in_=x_d,
                        in_offset=bass.IndirectOffsetOnAxis(ap=offs, axis=0),
                        bounds_check=T - 1, oob_is_err=False,
                    )
                    y2 = ypool.tile([128, GRP * D], f32)
                    for s in range(grp * GRP, (grp + 1) * GRP):
                        e = s // tpc
                        j = s - grp * GRP
                        g = g2[:, j * D:(j + 1) * D]

                        if K_TRANS == "bf16":
                            gb = gbpool.tile([128, D], bf16)
                            nc.vector.tensor_copy(gb[:], g)
                            gt_ps = psGT.tile([128, D], bf16)
                            tsrc, tident = gb[:], ident
                        else:
                            gt_ps = psGT.tile([128, D], f32)
                            tsrc, tident = g, ident_f
                        for k in range(8):
                            nc.tensor.transpose(
                                out=gt_ps[:, k * 128:(k + 1) * 128],
                                in_=tsrc[:, k * 128:(k + 1) * 128],
                                identity=tident[:],
                            )
                        gt_sb = gtpool.tile([128, D], mmdt)
                        nc.scalar.copy(gt_sb[:, 0:512], gt_ps[:, 0:512])
                        nc.scalar.copy(gt_sb[:, 512:1024], gt_ps[:, 512:1024])

                        d_ps = psD.tile([128, 128], f32)
                        for k in range(8):
                            nc.tensor.matmul(
                                d_ps[:],
                                lhsT=wdt_sb[:, e * 1024 + k * 128: e * 1024 + (k + 1) * 128],
                                rhs=gt_sb[:, k * 128:(k + 1) * 128],
                                start=(k == 0), stop=(k == 7),
                            )
                        d_sb = dpool.tile([128, 128], mmdt)
                        nc.scalar.copy(d_sb[:], d_ps[:])

                        u_ps = psU.tile([128, D], f32)
                        nc.tensor.matmul(u_ps[:, 0:512], lhsT=d_sb[:],
                                         rhs=wut_sb[:, e * 1024: e * 1024 + 512],
                                         start=True, stop=True)
                        nc.tensor.matmul(u_ps[:, 512:1024], lhsT=d_sb[:],
                                         rhs=wut_sb[:, e * 1024 + 512: e * 1024 + 1024],
                                         start=True, stop=True)

                        nc.vector.tensor_add(y2[:, j * D:(j + 1) * D], u_ps[:], g)

                    nc.gpsimd.indirect_dma_start(
                        out=y_d, out_offset=bass.IndirectOffsetOnAxis(ap=offs, axis=0),
                        in_=y2[:], in_offset=None,
                        bounds_check=T - 1, oob_is_err=False,
                    )

    nc.compile()
    return nc


def _prep_weights(Wd: np.ndarray, Wu: np.ndarray):
    import ml_dtypes
    wdt = np.zeros((128, NE, 8, 128), dtype=np.float32)
    wdt[:, :7] = Wd.reshape(7, 128, 8, 128).transpose(3, 0, 2, 1)
    wut = np.zeros((128, NE, 1024), dtype=np.float32)
    wut[:, :7] = Wu.transpose(2, 0, 1)
    odt = ml_dtypes.bfloat16 if K_MMDT == "bf16" else np.float32
    return (np.ascontiguousarray(wdt.reshape(128, NE * 1024)).astype(odt),
            np.ascontiguousarray(wut.reshape(128, NE * 1024)).astype(odt))


def _pick_cap(router_indices: np.ndarray, LRA_mask: np.ndarray) -> int:
    tok = router_indices[..., 0]
    key_active = np.zeros(8, dtype=bool)
    key_active[np.clip(LRA_mask, 0, 7)] = True
    active = key_active[tok] & (tok != 7)
    e_eff = np.where(active, tok, 7)
    mx = 1
    for b in range(e_eff.shape[0]):
        mx = max(mx, int(np.bincount(e_eff[b], minlength=8).max()))
    cap = ((mx + 127) // 128) * 128
    # keep slot count PAIR-aligned
    while (NE * cap // 128) % PAIR:
        cap += 128
    return cap


def kernel(x, router_indices, LRA_mask, Wd, Wu):
    x = np.asarray(x, dtype=np.float32)
    router_indices = np.asarray(router_indices, dtype=np.int32)
    LRA_mask = np.asarray(LRA_mask, dtype=np.int32)
    Wd = np.asarray(Wd, dtype=np.float32)
    Wu = np.asarray(Wu, dtype=np.float32)

    cap = _pick_cap(router_indices, LRA_mask)
    if cap not in _compiled:
        _compiled[cap] = _build_nc(cap)
    nc = _compiled[cap]

    wdt, wut = _prep_weights(Wd, Wu)
    in_maps = []
    for b in range(B):
        in_maps.append({
            "x": np.ascontiguousarray(x[b]),
            "ridx": np.ascontiguousarray(router_indices[b]),
            "lra": LRA_mask,
            "wdt": wdt,
            "wut": wut,
        })
    res = bass_utils.run_bass_kernel_spmd(nc, in_maps, core_ids=list(range(B)))
    out = np.stack([res.results[b]["y"] for b in range(B)], axis=0)
    return out.astype(np.float32)


if __name__ == "__main__":
    rng = np.random.default_rng(0)
    x = rng.standard_normal((B, T, D), dtype=np.float32)
    ridx = rng.integers(0, 8, size=(B, T, 1), dtype=np.int32)
    lra = np.arange(7, dtype=np.int32)
    Wd = (rng.standard_normal((7, 128, 1024)) * 0.01).astype(np.float32)
    Wu = (rng.standard_normal((7, 1024, 128)) * 0.01).astype(np.float32)
    y = kernel(x, ridx, lra, Wd, Wu)
    print("out", y.shape, y.dtype)
